# revision 30
# baseline (speedup 1.0000x reference)
"""Trainium2 Bass kernel for nn_BioSimulator.

Math: out[b,h,w] = clip(2 * sum_n Bw[b,n] * exp(-((px-vx[n])^2+(py-vy[n])^2)
                        * deg2pix^2 / (2*sigma_px[b,n]^2)), 0, 1)

px varies only along w and py only along h, so the Gaussian separates:
    exp(-(dx^2+dy^2)*c) = exp(-dx^2*c) * exp(-dy^2*c)
and the sum over points becomes a matmul over the point axis (transposed
output: stationary = Gx window, moving = 2Bw*Gy window).

sigma_px <= 2.01 px for this parameterization, so a Gaussian's support is
< +-14 px around its center (vx, vy).  That makes the problem windowable:

  - The 1024 points are SORTED by w-pixel position and sharded into
    quartiles (batch 2 x quartile 4 = 8 cores).  Every quartile spans
    <= 95 padded pixels -> each core touches a single 96-column w-window.
  - Within a quartile, points are sorted by h-pixel position and split into
    two 128-point tiles.  Every such tile spans <= 141 padded pixels -> each
    tile touches a 144-column h-window.

Each tile is then a matmul [Gx-window 96]^T @ [2Bw*Gy-window 144] into PSUM,
and the host pastes the per-tile banks at their h-offsets while summing
shards (overlapping h-windows just add, exactly like shards).  Pixels
outside every window receive < 3e-11 per point in the reference and are
exactly 0 here (the output is later clipped to [0,1] anyway).

Per-core device program (all per-point math is baked into the input table on
the host, so the device has NO scalar prep and NO small DMAs):
  - ONE input DMA: sq [128, 480] bf16 -- per tile p (cols 240p..240p+239),
    point row n (h0 = the tile's h-window start):
      sq[n, 240p + j]      = negc * ((xs[w0+j]-vx)*d2p)^2             (Gx)
      sq[n, 240p + 96 + j] = negc * ((ys[h0+j]-vy)*d2p)^2 + ln(2*Bw)  (Gy)
    with negc = -0.5/max(sigma_px^2, 1).  The exponent is bf16: its rounding
    is RELATIVE to the exponent value, so the Gaussian error stays ~0.4%
    where it matters (full-pipeline rel_l2 vs reference: ~1.7e-3).
  - ACT: exp per tile [128, 240] bf16 -> bf16 (Gx | 2Bw*Gy fused by the
    bake).  The auto-inserted exp-table load (~1.3us, no data deps) runs
    during the input-DMA latency and is the critical-path head.
  - PE: bf16 matmuls (bf16 keeps full rate below 256 moving columns,
    unlike f32r): tile 0 -> one bank; tile 1 -> two banks (h sub-halves).
  - DVE copies tile-0's bank while tile 1 is still in ACT/PE; tile-1's two
    banks are copied concurrently by DVE and ACT.  Separate banks per copy:
    PSUM reads of one tile serialize (destructive-read hazard).
  - Output via a PREPARED SWDGE scatter (dma_scatter_add prepare_only +
    trigger_dma): descriptors are generated on the idle Pool engine early in
    the kernel, so the post-compute tail is just trigger + transfer + sem --
    skipping the ~1.3us HWDGE-issue + DGE-delay chain a plain dma_start
    pays.  Scatter rows are 288 bytes (>512B rows double-add
    nondeterministically on HW -- they exceed a single SDMA packet).
    Indices (identity) are built on-device with memset+iota.
"""

import numpy as np
import ml_dtypes

import concourse.bass as bass
import concourse.bacc as bacc
import concourse.mybir as mybir
from concourse import tile
from concourse.bass_utils import run_bass_kernel_spmd

N_CORES = 8
NSHARDS = 4        # point shards (w-quartiles) per batch
PPC = 256          # points per core
NPT = 128          # points per partition tile
B = 2
H = W = 256
WIN = 96           # output w-window per core (every quartile spans <= 95)
HWIN = 144         # output h-window per point tile
MARGIN = 14.0      # px; exp(-0.5*(14/2.01)^2) ~ 3e-11, below bf16 noise
CW = WIN + HWIN    # table columns per tile
DUMP = 2 * WIN     # partial dump row for the unused scatter tokens

SPREAD = 0.000675
R2S = 0.5
SLOPE = 19152642.5
HALF = 1.057e-07
RHEO = 2.39e-05
FREQ = 300.0
PW = 0.00017
I_SCALE = 8e-05

F32 = mybir.dt.float32
BF16 = mybir.dt.bfloat16
I16 = mybir.dt.int16
ACT = mybir.ActivationFunctionType

_NC = None


def _build_nc():
    nc = bacc.Bacc(None, target_bir_lowering=False, debug=False,
                   num_devices=N_CORES)
    sq = nc.dram_tensor("sq", [NPT, 2 * CW], BF16, kind="ExternalInput")
    # partial[t*96 + p, j] (j < HWIN): contribution to out[b, h0[t]+j, w0+p].
    # Rows are padded to 256 columns (the scatter's row stride must be a
    # multiple of 256 bytes); row DUMP swallows the unused scatter tokens.
    partial = nc.dram_tensor("partial", [2 * WIN + 1, H], BF16,
                             kind="ExternalOutput")

    with tile.TileContext(nc) as tc:
        with (
            tc.tile_pool(name="const", bufs=1) as cpool,
            tc.tile_pool(name="psum", bufs=2, space="PSUM") as psum,
        ):
            # Scatter indices, [16, 16] int16 wrapped: token i lives at
            # [i%16, i//16].  Tokens 0-95 -> rows 0-95 (tile-0 bank),
            # 128-223 -> rows 96-191 (tile-1 bank); tokens 96-127 and
            # 224-255 read unused (zeroed) ob partitions and land on the
            # DUMP row, keeping all 256 tokens valid.
            idxt = cpool.tile([NPT, 16], I16)
            nc.gpsimd.memset(idxt[:], DUMP)
            nc.gpsimd.iota(idxt[0:16, 0:6], pattern=[[16, 6]], base=0,
                           channel_multiplier=1)
            nc.gpsimd.iota(idxt[0:16, 8:14], pattern=[[16, 6]], base=WIN,
                           channel_multiplier=1)

            # ob[p, t, j]: scatter token i reads [i%128, i//128, :]; only
            # partitions 0-95 carry bank rows, the rest feed the DUMP row
            # (zeroed so the reads are defined).
            ob = cpool.tile([NPT, 2, HWIN], BF16)
            nc.gpsimd.memset(ob[WIN:NPT, :, :], 0)

            sqt = cpool.tile([NPT, 2 * CW], BF16)
            nc.sync.dma_start(sqt[:], sq[:])

            gxy = [cpool.tile([NPT, CW], BF16, name=f"gxy{p}")
                   for p in range(2)]
            # Tile 0: one matmul into one bank, copied by DVE while tile 1
            # is still in ACT/PE.  Tile 1: two matmuls into two separate
            # banks (h sub-halves, same total PE cycles) so its copy can be
            # split across DVE and ACT -- PSUM reads of a single tile are
            # serialized by the framework (destructive-read hazard), two
            # banks run concurrently.  The split (85/59) balances DVE
            # (1.04ns/col + 125) against ACT (0.83ns/col + 164).
            HA = 85
            ps0 = psum.tile([WIN, H], F32, tag="ps0", name="ps0")
            ps1a = psum.tile([WIN, H], F32, tag="ps1a", name="ps1a")
            ps1b = psum.tile([WIN, H], F32, tag="ps1b", name="ps1b")
            nc.scalar.activation(gxy[0][:], sqt[:, 0:CW], ACT.Exp)
            nc.tensor.matmul(ps0[:, 0:HWIN], gxy[0][:, 0:WIN],
                             gxy[0][:, WIN:CW], start=True, stop=True)
            nc.scalar.activation(gxy[1][:], sqt[:, CW:2 * CW], ACT.Exp)
            nc.tensor.matmul(ps1a[:, 0:HA], gxy[1][:, 0:WIN],
                             gxy[1][:, WIN:WIN + HA], start=True, stop=True)
            nc.tensor.matmul(ps1b[:, 0:HWIN - HA], gxy[1][:, 0:WIN],
                             gxy[1][:, WIN + HA:CW], start=True, stop=True)
            nc.vector.tensor_copy(ob[0:WIN, 0, :], ps0[:, 0:HWIN])
            nc.vector.tensor_copy(ob[0:WIN, 1, 0:HA], ps1a[:, 0:HA])
            nc.scalar.copy(ob[0:WIN, 1, HA:HWIN], ps1b[:, 0:HWIN - HA])
            # Prepared scatter: EMITTED after ob's writers so Tile records
            # the RAW edges (and defers them to the trigger), but it EXECUTES
            # early -- its only sync dep is the idx tile, so descriptor
            # generation runs on the idle Pool engine during the input-DMA
            # latency.  The post-compute tail is then just trigger+transfer.
            dma_sem = nc.alloc_semaphore("swdge_dma")
            nc.gpsimd.dma_scatter_add(
                partial[:, 0:HWIN], ob[:], idxt[:],
                2 * NPT, 2 * NPT, HWIN, elem_step=H,
                prepare_only=True, sem=dma_sem,
            )
            nc.gpsimd.trigger_dma(count=None)
    nc.compile()
    return nc


def _get_nc():
    global _NC
    if _NC is None:
        _NC = _build_nc()
    return _NC


def _plan(vx, vy, px):
    """Sort points by w-pixel into quartiles; vy-sort tiles inside each;
    pick each core's w-window and each tile's h-window."""
    fov = px.max()
    d2p = W / (fov * 2.0)
    wx = (vx + fov) * d2p
    wy = (vy + fov) * d2p
    order = np.argsort(wx)

    def window(pos, width):
        # Support clipped to the screen: off-screen Gaussian mass has no
        # output pixels, so only [0, W) needs covering.
        lo = max(0, int(np.floor(pos.min() - MARGIN)))
        hi = min(W, int(np.ceil(pos.max() + MARGIN)) + 1)
        assert hi - lo <= width, (lo, hi, width)
        start = min(lo, W - width)
        assert start <= lo and hi <= start + width, (lo, hi, start)
        return start

    sels, w0s, h0s = [], [], []
    for s in range(NSHARDS):
        q = order[s * PPC:(s + 1) * PPC]
        w0s.append(window(wx[q], WIN))
        q = q[np.argsort(wy[q])]
        tiles, th0 = [], []
        for t in range(2):
            sel = q[t * NPT:(t + 1) * NPT]
            tiles.append(sel)
            th0.append(window(wy[sel], HWIN))
        sels.append(tiles)
        h0s.append(th0)
    return sels, w0s, h0s, d2p


def make_in_maps(stimulation, vx, vy, M, px, py, idx):
    stimulation = np.asarray(stimulation, dtype=np.float64)
    vx = np.asarray(vx, dtype=np.float64)
    vy = np.asarray(vy, dtype=np.float64)
    M = np.asarray(M, dtype=np.float64)
    px = np.asarray(px, dtype=np.float64)
    py = np.asarray(py, dtype=np.float64)
    idx = np.asarray(idx)

    sels, w0s, h0s, d2p = _plan(vx, vy, px)
    xs = px[0, :]            # px[h,w] = xs[w]
    ys = py[:, 0]            # py[h,w] = ys[h]
    flat = stimulation.reshape(B, -1)[:, idx]          # [B, N]
    I = flat * I_SCALE
    Bw = 1.0 / (1.0 + np.exp(-SLOPE * (np.maximum(I - RHEO, 0.0) * PW * FREQ
                                       - HALF)))
    sig2px = np.maximum((I / SPREAD) * (R2S * d2p / M[None, :]) ** 2, 1.0)
    negc = -0.5 / sig2px                               # [B, N]
    ln2bw = np.log(2.0 * Bw)                           # [B, N]

    in_maps = []
    for c in range(N_CORES):
        b, s = divmod(c, NSHARDS)
        w0 = w0s[s]
        sq = np.empty((NPT, 2 * CW), np.float64)
        for p in range(2):
            sel = sels[s][p]
            h0 = h0s[s][p]
            nc_ = negc[b, sel][:, None]
            sq[:, CW * p:CW * p + WIN] = (
                nc_ * ((xs[None, w0:w0 + WIN] - vx[sel, None]) * d2p) ** 2)
            sq[:, CW * p + WIN:CW * (p + 1)] = (
                nc_ * ((ys[None, h0:h0 + HWIN] - vy[sel, None]) * d2p) ** 2
                + ln2bw[b, sel][:, None])
        in_maps.append({"sq": sq.astype(ml_dtypes.bfloat16)})
    return in_maps


def combine(results, w0s, h0s):
    acc = np.zeros((B, H, W), np.float32)
    for c, r in enumerate(results):
        b, s = divmod(c, NSHARDS)
        w0 = w0s[s]
        part = np.asarray(r["partial"])[:, 0:HWIN].astype(np.float32)
        for t in range(2):
            h0 = h0s[s][t]
            # partial[t*128+p, j] -> out[b, h0+j, w0+p]
            acc[b, h0:h0 + HWIN, w0:w0 + WIN] += part[t * WIN:(t + 1) * WIN].T
    return np.clip(acc, 0.0, 1.0)[:, None, :, :].astype(np.float32)


def kernel(stimulation, vx, vy, M, px, py, idx):
    nc = _get_nc()
    in_maps = make_in_maps(stimulation, vx, vy, M, px, py, idx)
    _, w0s, h0s, _ = _plan(np.asarray(vx, np.float64),
                           np.asarray(vy, np.float64),
                           np.asarray(px, np.float64))
    res = run_bass_kernel_spmd(nc, in_maps, list(range(N_CORES)))
    return combine(res.results, w0s, h0s)
